# revision 38
# baseline (speedup 1.0000x reference)
"""CTRNN with per-sample Hebbian plasticity on 8 Trainium2 NeuronCores.

Data-parallel over the sample axis N: each core owns N/8 = 32 samples and
runs the full T-step scan locally; parameters are replicated.

Algorithm (per core). The effective recurrent input is
  rec_t = r_t @ (a*W_rec) + sum_h r_t[n,h] * (a*c*hebb_t)[n,h,k].
The scaled trace A' = sum_{j} gamma_j * r_j (x) r_{j+1} (with the (1-eta)
decay absorbed into gamma_j, "scaled tracking") is kept STALE by up to
W steps in SBUF.  The missing recent rank-1 terms are applied as
attention-style corrections in rows layout: dot products via DVE
tensor_tensor_reduce, per-sample axpy via tensor_scalar with a
per-partition scalar, then a PE transpose-accumulate into the rec PSUM
tile.  Every W steps the window's rank-W update folds into A' with one
K=W bf16 matmul per sample whose operand stacks come from per-sample
strided PE transposes of the tanh-history buffer RT (gamma scaling applied
during the PSUM->SBUF copy via a per-partition scale table).  There are no
DMAs and no departition moves inside the scan.

Host/wire path: the axon tunnel (~35-50 MB/s per direction, partially
duplex) dominates wall-clock — the device kernel itself is ~2 ms — so
the runtime
  * keeps jitted shard_map(bass_exec) executables alive across calls
    (no per-call retrace/re-lower) and replicated weights device-resident;
  * ships x row-major in fp16 (fp16's 11-bit mantissa keeps the chaotic
    recurrence's input-noise amplification at ~2.5e-3 output error where
    bf16 would be ~1.8e-2; fp8 diverges to 0.29) — PE transposes it on
    device;
  * returns y quantized to int8 with a per-core, per-output-dim scale
    computed on device (max|y| per partition; ~4e-3 absmax-relative
    quantization error), halving the download;
  * splits the scan into pipeline STAGES at fold boundaries (multiples of
    W).  Stage state (h, Hebbian trace A, last-W tanh history) passes
    between stages as device-resident arrays with zero wire cost, so
    stage k's y download overlaps stage k+1's x upload + execution.
No zero output buffers are donated: the kernel fully writes its outputs.
"""

import numpy as np
from concurrent.futures import ThreadPoolExecutor
from contextlib import ExitStack

import jax
from jax.experimental.shard_map import shard_map
from jax.sharding import Mesh, PartitionSpec, NamedSharding

import concourse.bass as bass
import concourse.tile as tile
from concourse import bacc, mybir, masks
from concourse import bass2jax

F32 = mybir.dt.float32
F16 = mybir.dt.float16
BF16 = mybir.dt.bfloat16
I8 = mybir.dt.int8
AF = mybir.ActivationFunctionType
OP = mybir.AluOpType

T_FULL = 512
N_FULL = 256
I_DIM = 64
H0_DIM = 32
H = 128
O_DIM = 64
N_CORES = 8
NS = N_FULL // N_CORES  # 32 samples per core
G = 4                   # trace groups
GS = NS // G            # 8 samples per group
W = 8                   # fold window (steps)
N_STAGES = 4            # fallback stage count (boundaries snap to multiples of W)
# Graded stage sizes: small first stage so the first y download starts early,
# small last stage so the download tail is short; interior boundaries must be
# multiples of W.
_BOUNDS_512 = [0, 64, 192, 320, 448, 511]
# Input noise injected late in the scan has little time to amplify through
# the chaotic recurrence (measured on the reference: int8 x from t=384 costs
# 3.9e-3 output error vs 2.5e-3 for fp16-everywhere, while int8 from t=0
# would diverge to 0.29).  Stages starting at or after this step take x in
# int8 with a host-computed global scale, saving upload bytes.
_X_INT8_FROM = 320


def _stage_bounds(S: int, k: int) -> list[int]:
    if S == T_FULL - 1:
        return list(_BOUNDS_512)
    bounds = [0]
    for j in range(1, k):
        b = (S * j // k) // W * W
        if b > bounds[-1]:
            bounds.append(b)
    bounds.append(S)
    return bounds


def build_stage(a: float, e: float, c: float, T: int, s0: int, s1: int):
    """One pipeline stage covering scan steps [s0, s1) of the full scan.

    Absolute-step indexing (beta, gtab, fold windows) is preserved, so
    concatenating stages reproduces the monolithic scan exactly.
    """
    S = T - 1
    first, last = s0 == 0, s1 == S
    SS = s1 - s0                      # steps in this stage
    XC = SS * NS                      # x columns
    YC = (SS + (1 if last else 0)) * NS  # y columns
    RC = (W + SS + (1 if last else 0)) * NS  # tanh-history columns (W prefix)
    NW = max((S - 1) // W, 1)         # folds in the FULL scan (gtab width)

    x_int8 = s0 >= _X_INT8_FROM

    nc = bacc.Bacc("TRN2", target_bir_lowering=False, debug=False)

    x_d = nc.dram_tensor("x", [XC, I_DIM], I8 if x_int8 else F16,
                         kind="ExternalInput").ap()
    if x_int8:
        xscl_d = nc.dram_tensor("xscl", [128, 1], F32, kind="ExternalInput").ap()
    if first:
        h0_d = nc.dram_tensor("h0", [H0_DIM, NS], F32, kind="ExternalInput").ap()
        wh0_d = nc.dram_tensor("w_h0", [H0_DIM, H], F32, kind="ExternalInput").ap()
        bh0_d = nc.dram_tensor("b_h0", [H, 1], F32, kind="ExternalInput").ap()
    else:
        hin_d = nc.dram_tensor("h_in", [H, NS], F32, kind="ExternalInput").ap()
        ain_d = nc.dram_tensor("a_in", [128, G * GS * H], BF16,
                               kind="ExternalInput").ap()
        rtp_d = nc.dram_tensor("rt_in", [128, W * NS], F32,
                               kind="ExternalInput").ap()
    win_d = nc.dram_tensor("w_in", [I_DIM, H], F16, kind="ExternalInput").ap()
    wrec_d = nc.dram_tensor("w_rec", [H, H], F32, kind="ExternalInput").ap()
    brec_d = nc.dram_tensor("b_rec", [H, 1], F32, kind="ExternalInput").ap()
    wout_d = nc.dram_tensor("w_out", [H, O_DIM], F32, kind="ExternalInput").ap()
    gt_d = nc.dram_tensor("gtab", [128, NW], F32, kind="ExternalInput").ap()
    y_d = nc.dram_tensor("y", [O_DIM, YC], I8, kind="ExternalOutput").ap()
    yscl_d = nc.dram_tensor("yscl", [O_DIM, 1], F32, kind="ExternalOutput").ap()
    if not last:
        hout_d = nc.dram_tensor("h_out", [H, NS], F32, kind="ExternalOutput").ap()
        aout_d = nc.dram_tensor("a_out", [128, G * GS * H], BF16,
                                kind="ExternalOutput").ap()
        rtl_d = nc.dram_tensor("rt_out", [128, W * NS], F32,
                               kind="ExternalOutput").ap()

    with tile.TileContext(nc) as tc, ExitStack() as ctx:
        const = ctx.enter_context(tc.tile_pool(name="const", bufs=1))
        big = ctx.enter_context(tc.tile_pool(name="big", bufs=1))

        ident = const.tile([128, 128], F32)
        masks.make_identity(nc, ident[:])
        ident16 = const.tile([128, 128], F16)
        nc.scalar.activation(ident16[:], ident[:], AF.Copy)
        w_rec = const.tile([H, H], F32)
        nc.sync.dma_start(w_rec[:], wrec_d)
        w_in = const.tile([I_DIM, H], F16)
        nc.sync.dma_start(w_in[:], win_d)
        w_out = const.tile([H, O_DIM], F32)
        nc.sync.dma_start(w_out[:], wout_d)
        b_rec = const.tile([H, 1], F32)
        nc.sync.dma_start(b_rec[:], brec_d)
        gtab = const.tile([128, NW], F32)
        nc.sync.dma_start(gtab[:], gt_d)

        U = big.tile([128, XC], F32)       # a*(x@W_in + b_rec), [k, (i, n)]
        RT = big.tile([128, RC], F32)      # tanh(h): W prefix slabs + stage slabs
        RT3b = RT.rearrange("p (t n) -> p n t", n=NS)
        A = [big.tile([128, GS * H], BF16, name=f"A{g}", tag=f"A{g}")
             for g in range(G)]            # scaled trace, [h, (n_in_group, k)]

        hh = ctx.enter_context(tc.tile_pool(name="hh", bufs=2))
        with tc.tile_pool(name="pro", bufs=3) as pro, \
             tc.tile_pool(name="pro_ps", bufs=2, space="PSUM") as pro_ps:
            if first:
                for g in range(G):
                    nc.vector.memset(A[g][:], 0.0)
                w_h0 = pro.tile([H0_DIM, H], F32, tag="wh0")
                nc.sync.dma_start(w_h0[:], wh0_d)
                b_h0 = pro.tile([H, 1], F32, tag="bh0")
                nc.sync.dma_start(b_h0[:], bh0_d)
                h0t = pro.tile([H0_DIM, NS], F32, tag="h0t")
                nc.sync.dma_start(h0t[:], h0_d)
                h0ps = pro_ps.tile([H, NS], F32, tag="h0ps")
                nc.tensor.matmul(h0ps[:], lhsT=w_h0[:], rhs=h0t[:],
                                 start=True, stop=True)
                h_cur = hh.tile([H, NS], F32, tag="h")
                nc.scalar.activation(h_cur[:], h0ps[:], AF.Identity,
                                     bias=b_h0[:, 0:1])
            else:
                h_cur = hh.tile([H, NS], F32, tag="h")
                nc.sync.dma_start(h_cur[:], hin_d)
                for g in range(G):
                    nc.sync.dma_start(A[g][:],
                                      ain_d[:, g * GS * H:(g + 1) * GS * H])
                nc.sync.dma_start(RT[:, 0:W * NS], rtp_d)

            if x_int8:
                xsc = pro.tile([128, 1], F32, tag="xsc")
                nc.sync.dma_start(xsc[:], xscl_d)
            # U = (x @ W_in + b_rec)^T (x arrives row-major fp16 or int8;
            # int8 is dequantized via a per-partition scale, PE transposes)
            r0 = 0
            while r0 < XC:
                rows_n = min(128, XC - r0)
                if x_int8:
                    xq = pro.tile([128, I_DIM], I8, tag="xq")
                    nc.sync.dma_start(xq[:rows_n, :], x_d[r0:r0 + rows_n, :])
                    xn = pro.tile([128, I_DIM], F16, tag="xn")
                    nc.scalar.activation(xn[:rows_n, :], xq[:rows_n, :],
                                         AF.Copy, scale=xsc[:rows_n, 0:1])
                else:
                    xn = pro.tile([128, I_DIM], F16, tag="xn")
                    nc.sync.dma_start(xn[:rows_n, :], x_d[r0:r0 + rows_n, :])
                xtp = pro_ps.tile([I_DIM, 128], F16, tag="xtp")
                nc.tensor.transpose(xtp[:, :rows_n], xn[:rows_n, :],
                                    ident16[:rows_n, :rows_n])
                xt = pro.tile([I_DIM, 128], F16, tag="xt")
                nc.scalar.activation(xt[:, :rows_n], xtp[:, :rows_n], AF.Copy)
                ups = pro_ps.tile([H, 128], F32, tag="ups")
                nc.tensor.matmul(ups[:, :rows_n], lhsT=w_in[:], rhs=xt[:, :rows_n],
                                 start=True, stop=True)
                nc.scalar.activation(U[:, r0:r0 + rows_n], ups[:, :rows_n],
                                     AF.Identity, bias=b_rec[:, 0:1])
                r0 += rows_n

        # ---- main scan over absolute steps [s0, s1) ----
        rows = {}
        with tc.tile_pool(name="sm", bufs=2) as sm, \
             tc.tile_pool(name="rr", bufs=W + 2) as rr, \
             tc.tile_pool(name="st", bufs=3) as st, \
             tc.tile_pool(name="ps_rec", bufs=2, space="PSUM") as ps_rec, \
             tc.tile_pool(name="ps_tr", bufs=1, space="PSUM") as ps_tr, \
             tc.tile_pool(name="ps_corr", bufs=1, space="PSUM") as ps_corr, \
             tc.tile_pool(name="ps_fold", bufs=1, space="PSUM") as ps_fold, \
             tc.tile_pool(name="ps_st", bufs=1, space="PSUM") as ps_st:
            for i in range(s0, s1):
                beta = (1.0 - e) ** i
                li = i - s0 + W               # local slab index in RT
                cur = slice(li * NS, (li + 1) * NS)
                slab_i = RT[:, cur]
                nc.scalar.activation(slab_i, h_cur[:], AF.Tanh)       # r_i
                trp = ps_tr.tile([NS, H], F32, tag="trp")
                nc.tensor.transpose(trp[:], slab_i, ident[:, :])
                rows[i] = rr.tile([NS, H], BF16, name="rows", tag="rows")
                nc.scalar.activation(rows[i][:], trp[:], AF.Copy)

                # fold the last W rank-1 terms into A every W steps.
                if i % W == 0 and i > 0:
                    jl, m = i - s0, i // W    # local t-offset of absolute j=i-W
                    for g in range(G):
                        ns0 = g * GS
                        stgL = st.tile([128, GS * 32], F32, tag="stgL")
                        nc.vector.memset(stgL[:], 0.0)
                        stgLv = stgL.rearrange("p (q w) -> p q w", w=32)
                        nc.scalar.activation(
                            stgLv[:, :, 0:W], RT3b[:, ns0:ns0 + GS, jl:jl + W],
                            AF.Copy)
                        stgR = st.tile([128, GS * 32], F32, tag="stgR")
                        nc.vector.memset(stgR[:], 0.0)
                        stgRv = stgR.rearrange("p (q w) -> p q w", w=32)
                        nc.scalar.activation(
                            stgRv[:, :, 0:W],
                            RT3b[:, ns0:ns0 + GS, jl + 1:jl + W + 1],
                            AF.Copy)
                        fps = ps_fold.tile([128, GS * H], F32, tag="fold")
                        for q in range(GS):
                            stpL = ps_st.tile([32, H], F32, tag="stkL")
                            nc.tensor.transpose(stpL[:],
                                                stgL[:, q * 32:(q + 1) * 32],
                                                ident[:, :])
                            lhs_n = st.tile([32, H], BF16, tag="lhs")
                            nc.scalar.activation(lhs_n[:], stpL[:], AF.Copy,
                                                 scale=gtab[0:32, m - 1:m])
                            stpR = ps_st.tile([32, H], F32, tag="stkR")
                            nc.tensor.transpose(stpR[:],
                                                stgR[:, q * 32:(q + 1) * 32],
                                                ident[:, :])
                            rhs_n = st.tile([32, H], BF16, tag="rhs")
                            nc.scalar.activation(rhs_n[:], stpR[:], AF.Copy)
                            nc.tensor.matmul(fps[:, q * H:(q + 1) * H],
                                             lhsT=lhs_n[:], rhs=rhs_n[:],
                                             start=True, stop=True)
                        nc.vector.tensor_tensor(A[g][:], A[g][:], fps[:], OP.add)

                # rec = r @ (a*W_rec) [+ beta * per-sample r^T A] [+ corr]
                B = W * (i // W)
                njs = i - B
                do_mv = i >= W
                rec = ps_rec.tile([H, NS], F32, tag="rec")
                nc.tensor.matmul(rec[:], lhsT=w_rec[:], rhs=slab_i,
                                 start=True, stop=not do_mv)
                if do_mv:
                    rTs = sm.tile([H, NS], BF16, tag="rTs")
                    nc.vector.tensor_scalar(rTs[:], slab_i, beta, None, OP.mult)
                    for n in range(NS):
                        g, j = divmod(n, GS)
                        nc.tensor.matmul(rec[:, n:n + 1],
                                         lhsT=A[g][:, j * H:(j + 1) * H],
                                         rhs=rTs[:, n:n + 1],
                                         start=False,
                                         stop=(n == NS - 1))

                # corrections for unfolded steps j in [B, i)
                if njs > 0:
                    cps = ps_corr.tile([H, NS], F32, tag="corr")
                    for idx, j in enumerate(range(B, i)):
                        coef = a * c * e * (1.0 - e) ** (i - 1 - j)
                        jk = sm.tile([NS, H], BF16, tag="jk")
                        nc.vector.tensor_tensor(jk[:], rows[i][:], rows[j][:],
                                                OP.mult)
                        dj = sm.tile([NS, 1], F32, tag="dj")
                        nc.vector.tensor_reduce(dj[:], jk[:],
                                                axis=mybir.AxisListType.X,
                                                op=OP.add)
                        tmpj = sm.tile([NS, H], F32, tag="tmpj")
                        nc.vector.tensor_scalar(tmpj[:], rows[j + 1][:],
                                                dj[:, 0:1], coef,
                                                OP.mult, OP.mult)
                        nc.tensor.matmul(cps[:], lhsT=tmpj[:],
                                         rhs=ident[:NS, :NS], is_transpose=True,
                                         start=(idx == 0), stop=(idx == njs - 1))

                # h update
                t3 = sm.tile([H, NS], F32, tag="t3")
                nc.vector.tensor_tensor(t3[:], rec[:],
                                        U[:, (i - s0) * NS:(i - s0 + 1) * NS],
                                        OP.add)
                if njs > 0:
                    t4 = sm.tile([H, NS], F32, tag="t4")
                    nc.vector.tensor_tensor(t4[:], t3[:], cps[:], OP.add)
                else:
                    t4 = t3
                hsc = sm.tile([H, NS], F32, tag="hsc")
                nc.scalar.activation(hsc[:], h_cur[:], AF.Copy, scale=1.0 - a)
                h_new = hh.tile([H, NS], F32, tag="h")
                nc.vector.tensor_tensor(h_new[:], t4[:], hsc[:], OP.add)
                h_cur = h_new
                rows.pop(i - W - 1, None)

            if last:
                # final tanh into the slab for t = S
                fli = S - s0 + W
                nc.scalar.activation(RT[:, fli * NS:(fli + 1) * NS], h_cur[:],
                                     AF.Tanh)

        # ---- epilogue: y = RT^T @ W_out, quantized to int8 with a per-
        # output-dim scale (max|y| per partition), plus state hand-off ----
        ysb = big.tile([O_DIM, YC], F32, name="ysb")
        with tc.tile_pool(name="ep", bufs=3) as ep, \
             tc.tile_pool(name="ep_ps", bufs=2, space="PSUM") as ep_ps:
            if not last:
                nc.sync.dma_start(hout_d, h_cur[:])
                for g in range(G):
                    nc.sync.dma_start(aout_d[:, g * GS * H:(g + 1) * GS * H],
                                      A[g][:])
                nc.sync.dma_start(rtl_d, RT[:, SS * NS:(SS + W) * NS])
            mx = ep.tile([O_DIM, 1], F32, tag="mx")
            nc.vector.memset(mx[:], 0.0)
            r0 = 0
            while r0 < YC:
                cols = min(512, YC - r0)
                ops_ = ep_ps.tile([O_DIM, 512], F32, tag="eops")
                nc.tensor.matmul(ops_[:, :cols], lhsT=w_out[:],
                                 rhs=RT[:, W * NS + r0:W * NS + r0 + cols],
                                 start=True, stop=True)
                nc.scalar.activation(ysb[:, r0:r0 + cols], ops_[:, :cols],
                                     AF.Copy)
                mxc = ep.tile([O_DIM, 1], F32, tag="mxc")
                nc.vector.tensor_reduce(mxc[:], ysb[:, r0:r0 + cols],
                                        axis=mybir.AxisListType.X, op=OP.max,
                                        apply_absolute_value=True)
                nc.vector.tensor_tensor(mx[:], mx[:], mxc[:], OP.max)
                r0 += cols
            rcp = ep.tile([O_DIM, 1], F32, tag="rcp")
            nc.vector.reciprocal(rcp[:], mx[:])
            qsc = ep.tile([O_DIM, 1], F32, tag="qsc")
            nc.vector.tensor_scalar(qsc[:], rcp[:], 127.0, None, OP.mult)
            dqs = ep.tile([O_DIM, 1], F32, tag="dqs")
            nc.vector.tensor_scalar(dqs[:], mx[:], 1.0 / 127.0, None, OP.mult)
            nc.sync.dma_start(yscl_d, dqs[:])
            r0 = 0
            while r0 < YC:
                cols = min(512, YC - r0)
                q8 = ep.tile([O_DIM, 512], I8, tag="q8")
                nc.scalar.activation(q8[:, :cols], ysb[:, r0:r0 + cols],
                                     AF.Copy, scale=qsc[:, 0:1])
                nc.sync.dma_start(y_d[:, r0:r0 + cols], q8[:, :cols])
                r0 += cols

    nc.compile()
    return nc


def make_gtab(a, e, c, T):
    S = T - 1
    NW = max((S - 1) // W, 1)
    p = np.arange(128) % W
    j = (np.arange(NW)[None, :] * W + p[:, None]).astype(np.float64)
    return (a * c * e * (1.0 - e) ** (-(j + 1.0))).astype(np.float32)


class _StageRT:
    """Jitted shard_map(bass_exec) for one stage, reused across calls."""

    def __init__(self, nc, mesh, sharding):
        self.nc = nc
        self.mesh = mesh
        self.sharding = sharding

        pname = nc.partition_id_tensor.name if nc.partition_id_tensor else None
        in_names: list[str] = []
        in_avals: list[jax.core.ShapedArray] = []
        out_names: list[str] = []
        out_avals: list[jax.core.ShapedArray] = []
        for alloc in nc.m.functions[0].allocations:
            if not isinstance(alloc, mybir.MemoryLocationSet):
                continue
            name = alloc.memorylocations[0].name
            if alloc.kind == "ExternalInput":
                if name != pname:
                    in_names.append(name)
                    in_avals.append(jax.core.ShapedArray(
                        tuple(alloc.tensor_shape), mybir.dt.np(alloc.dtype)))
            elif alloc.kind == "ExternalOutput":
                out_names.append(name)
                out_avals.append(jax.core.ShapedArray(
                    tuple(alloc.tensor_shape), mybir.dt.np(alloc.dtype)))
        assert nc.dbg_addr is None, "runtime expects debug=False"
        self.in_names = in_names
        self.out_names = out_names

        # bass_exec's config.in_names must cover every operand: real inputs,
        # one placeholder per output (the NEFF never reads it — outputs are
        # fully written, so no zero-fill donation is needed and the same
        # device-resident dummy is reused every call), then partition-id.
        names_t = tuple(in_names) + tuple(out_names) + ((pname,) if pname else ())
        onames_t = tuple(out_names)
        oavals_t = tuple(out_avals)
        self.dummy_outs = [
            jax.device_put(
                np.zeros((N_CORES * av.shape[0], *av.shape[1:]), av.dtype),
                sharding)
            for av in out_avals
        ]

        def _body(*args):
            operands = list(args)
            if pname is not None:
                operands.append(bass2jax.partition_id_tensor())
            outs = bass2jax._bass_exec_p.bind(
                *operands,
                out_avals=oavals_t,
                in_names=names_t,
                out_names=onames_t,
                lowering_input_output_aliases=(),
                sim_require_finite=True,
                sim_require_nnan=True,
                nc=nc,
            )
            return tuple(outs)

        n_ops = len(in_names) + len(out_names)
        mapped = shard_map(
            _body, mesh=mesh,
            in_specs=(PartitionSpec("core"),) * n_ops,
            out_specs=(PartitionSpec("core"),) * len(out_names),
            check_rep=False)
        # AOT-compile on the effects-suppressed fast path (C++ dispatch);
        # fall back to a plain jit if that path is unavailable.
        try:
            abstract = [
                jax.ShapeDtypeStruct((N_CORES * av.shape[0], *av.shape[1:]),
                                     av.dtype, sharding=sharding)
                for av in (*in_avals, *out_avals)
            ]
            self.fn = bass2jax.fast_dispatch_compile(
                lambda: jax.jit(mapped, keep_unused=True)
                .lower(*abstract).compile())
        except Exception:
            self.fn = jax.jit(mapped, keep_unused=True)

    def run(self, by_name: dict):
        args = [by_name[n] for n in self.in_names] + self.dummy_outs
        return self.fn(*args)


class _Pipeline:
    def __init__(self, a: float, e: float, c: float, T: int):
        bass2jax.install_neuronx_cc_hook()
        S = T - 1
        self.bounds = _stage_bounds(S, N_STAGES)
        devices = jax.devices()[:N_CORES]
        assert len(devices) == N_CORES, \
            f"need {N_CORES} devices, have {len(devices)}"
        self.mesh = Mesh(np.asarray(devices), ("core",))
        self.sharding = NamedSharding(self.mesh, PartitionSpec("core"))
        self.stages = [
            _StageRT(build_stage(a, e, c, T, lo, hi), self.mesh, self.sharding)
            for lo, hi in zip(self.bounds[:-1], self.bounds[1:])
        ]
        self._wcache: dict = {}

    def put_replicated(self, name: str, arr: np.ndarray):
        """Device-resident replicated weight, re-uploaded only if changed."""
        cached = self._wcache.get(name)
        if (cached is not None and cached[0].shape == arr.shape
                and cached[0].dtype == arr.dtype and np.array_equal(cached[0], arr)):
            return cached[1]
        glob = np.concatenate([arr] * N_CORES, axis=0)
        dev = jax.device_put(glob, self.sharding)
        self._wcache[name] = (arr.copy(), dev)
        return dev


_PIPELINES: dict = {}
_POOL = ThreadPoolExecutor(max_workers=8)


def _get_pipeline(a, e, c, T) -> _Pipeline:
    key = (round(a, 9), round(e, 9), round(c, 9), T)
    if key not in _PIPELINES:
        _PIPELINES[key] = _Pipeline(a, e, c, T)
    return _PIPELINES[key]


def _x_chunk(input_ts, lo, hi):
    """x rows per core: G[c, s*NS+n, i] = input_ts[1+lo+s, NS*c+n, i].
    Core-major block move of contiguous (NS, I_DIM) tiles + one-pass cast.
    Stages at/after _X_INT8_FROM ship int8 with a global scale instead of
    fp16 (late-injected quantization noise barely amplifies)."""
    ss = hi - lo
    blk = (input_ts[1 + lo:1 + hi].reshape(ss, N_CORES, NS, I_DIM)
           .transpose(1, 0, 2, 3))
    if lo >= _X_INT8_FROM:
        s = float(np.abs(blk).max()) / 127.0
        if s == 0.0:
            s = 1.0
        tmp = blk * np.float32(1.0 / s)   # materializes C-order
        np.rint(tmp, out=tmp)
        q = tmp.astype(np.int8)           # |tmp| <= 127 by construction
        scl = np.full((N_CORES * 128, 1), s, np.float32)
        return q.reshape(N_CORES * ss * NS, I_DIM), scl
    return np.asarray(blk, dtype=np.float16,
                      order="C").reshape(N_CORES * ss * NS, I_DIM), None


def _y_chunk(out, y8, dq, lo, tk):
    """Dequantize int8 y (per-core, per-output-dim scale) into out[lo:lo+tk]."""
    yf = np.multiply(y8, dq, dtype=np.float32)  # [N_CORES*O_DIM, tk*NS]
    src = yf.reshape(N_CORES, O_DIM, tk, NS).transpose(2, 0, 3, 1)
    out[lo:lo + tk].reshape(tk, N_CORES, NS, O_DIM)[...] = src


def kernel(h0_data, input_ts, W_h0, b_h0, W_in, W_rec, b_rec,
           alpha_rec, W_out, alpha, eta):
    h0_data = np.asarray(h0_data, np.float32)
    input_ts = np.asarray(input_ts, np.float32)
    W_h0 = np.asarray(W_h0, np.float32)
    b_h0 = np.asarray(b_h0, np.float32)
    W_in = np.asarray(W_in, np.float32)
    W_rec = np.asarray(W_rec, np.float32)
    b_rec = np.asarray(b_rec, np.float32)
    alpha_rec = np.asarray(alpha_rec, np.float32)
    W_out = np.asarray(W_out, np.float32)
    a = float(np.asarray(alpha).reshape(-1)[0])
    e = float(np.asarray(eta).reshape(-1)[0])
    c = float(alpha_rec.reshape(-1)[0])
    assert np.allclose(alpha_rec, c), "kernel assumes uniform alpha_rec"

    T = input_ts.shape[0]
    S = T - 1
    pl = _get_pipeline(a, e, c, T)

    # kick off x transforms first — they gate upload enqueue
    spans = list(zip(pl.bounds[:-1], pl.bounds[1:]))
    x_futs = [_POOL.submit(_x_chunk, input_ts, lo, hi) for lo, hi in spans]

    weights = {
        "w_h0": pl.put_replicated("w_h0", W_h0),
        "b_h0": pl.put_replicated("b_h0", b_h0.reshape(H, 1)),
        "w_in": pl.put_replicated("w_in", (a * W_in).astype(np.float16)),
        "w_rec": pl.put_replicated("w_rec", a * W_rec),
        "b_rec": pl.put_replicated("b_rec", a * b_rec.reshape(H, 1)),
        "w_out": pl.put_replicated("w_out", W_out),
        "gtab": pl.put_replicated("gtab", make_gtab(a, e, c, T)),
    }
    # h0^T per core: G[c, h, n] = h0_data[0, NS*c+n, h]
    h0_g = np.ascontiguousarray(
        h0_data[0].reshape(N_CORES, NS, H0_DIM).transpose(0, 2, 1)
    ).reshape(N_CORES * H0_DIM, NS)

    # dispatch all stages; state flows on-device, y downloads overlap uploads
    ys = []
    state = None
    for k, (lo, hi) in enumerate(spans):
        by = dict(weights)
        by["x"], xscl = x_futs[k].result()
        if xscl is not None:
            by["xscl"] = xscl
        if k == 0:
            by["h0"] = h0_g
        else:
            by["h_in"], by["a_in"], by["rt_in"] = state
        stage = pl.stages[k]
        outs = dict(zip(stage.out_names, stage.run(by)))
        ys.append((outs["y"], outs["yscl"]))
        outs["y"].copy_to_host_async()
        outs["yscl"].copy_to_host_async()
        if "h_out" in outs:
            state = (outs["h_out"], outs["a_out"], outs["rt_out"])

    out = np.empty((T, N_FULL, O_DIM), np.float32)
    y_futs = []
    for k, (lo, hi) in enumerate(spans):
        tk = hi - lo + (1 if hi == S else 0)
        y8 = np.asarray(ys[k][0])   # [N_CORES * O_DIM, tk * NS] int8
        dq = np.asarray(ys[k][1])   # [N_CORES * O_DIM, 1] f32
        y_futs.append(_POOL.submit(_y_chunk, out, y8, dq, lo, tk))
    for f in y_futs:
        f.result()
    return out
